# revision 1
# baseline (speedup 1.0000x reference)
"""Trainium2 Bass kernel: segment-mean over token segments + pairwise-diff edge MLP.

Reference computation (per batch row b):
  seg = cumsum(ids == 3); valid = ids != 3
  means[n] = mean of features[s] over tokens with seg==n & valid (n < 8), 0-count -> sum/1
  diff[i,j] = means[i] - means[j]                          # [8,8,H]
  out[i,j]  = relu(relu(diff @ W1 + b1) @ Wm + bm) @ W2 + b2   # [8,8,150]

Distribution: data-parallel over batch B=128 across 8 NeuronCores (16 rows/core),
tiny MLP weights replicated, no cross-core communication.

Device algorithm per core:
  stage1: means^T-ish  [8seg, 768] per row via TensorE: onehot (stationary, 0/1,
          host-precomputed) x features (moving) accumulated over 8 token chunks,
          scaled by 1/count on PSUM->SBUF eviction (ScalarE activation scale).
  diff:   one matmul per (group-of-4-rows, h-chunk): diffT = means^T @ E4 where E4
          is a constant +-1 selection matrix -> fuses the transpose AND the
          pairwise difference. Output columns = (g2, b2, i, j) = 256 per 4 rows.
  MLP:    transposed matmuls, contraction dim on partitions, c-dim split 128+22.
          Biases b1/bm applied as per-partition activation bias (c on partitions);
          b2 added via a K=1 matmul with a ones row. Final out is [rows, 150].
"""

import sys

import numpy as np

if "/opt/trn_rl_repo" not in sys.path:
    sys.path.insert(0, "/opt/trn_rl_repo")

import concourse.bass as bass
import concourse.mybir as mybir
from concourse import bacc
from concourse.bass import ds
from concourse.bass_utils import run_bass_kernel_spmd
from concourse.tile import TileContext

B, S, H, C = 128, 1024, 768, 150
NSEG = 8
SEP_ID = 3
NCORES = 8
RPC = B // NCORES  # 16 rows per core
TCH = S // 128     # 8 token chunks
HC = H // 128      # 6 hidden chunks
HHALF = 384        # H split for PSUM bank limit
CC = ((0, 128), (128, 22))  # c-dim (150) chunks
CPAD = 256         # final free dim padded so fp32r runs full-rate

F32 = mybir.dt.float32
F32R = mybir.dt.float32r

# fp32r = single-pass fp32 matmul mode (reduced internal precision, 4x faster
# moving-dim throughput when free dim >= 256). Flags allow fp32 fallback.
F32R_STAGE1 = True
F32R_MLP = True


def build_program(rpc=RPC, tch=TCH, f32r_stage1=F32R_STAGE1, f32r_mlp=F32R_MLP,
                  feat_bufs=4):
    S_ = tch * 128
    ngp = rpc // 4  # group-pairs: 4 batch rows -> 256 output rows each
    nc = bass.Bass("TRN2", target_bir_lowering=False, debug=False)

    DT1 = F32R if f32r_stage1 else F32   # stage-1 matmul operand dtype
    DTM = F32R if f32r_mlp else F32      # MLP matmul operand dtype
    feats_d = nc.dram_tensor("features", [rpc, S_, H], DT1, kind="ExternalInput").ap()
    ohT_d = nc.dram_tensor("ohT", [128, rpc * tch * NSEG], DT1, kind="ExternalInput").ap()
    icnt_d = nc.dram_tensor("icnt", [NSEG, rpc], F32, kind="ExternalInput").ap()
    w1p_d = nc.dram_tensor("w1p", [128, HC * C], DTM, kind="ExternalInput").ap()
    wm0_d = nc.dram_tensor("wm0", [128, C], DTM, kind="ExternalInput").ap()
    wm1_d = nc.dram_tensor("wm1", [22, C], DTM, kind="ExternalInput").ap()
    w20_d = nc.dram_tensor("w20", [128, CPAD], DTM, kind="ExternalInput").ap()
    w21_d = nc.dram_tensor("w21", [22, CPAD], DTM, kind="ExternalInput").ap()
    b1c0_d = nc.dram_tensor("b1c0", [128, 1], F32, kind="ExternalInput").ap()
    b1c1_d = nc.dram_tensor("b1c1", [22, 1], F32, kind="ExternalInput").ap()
    bm0_d = nc.dram_tensor("bm0", [128, 1], F32, kind="ExternalInput").ap()
    bm1_d = nc.dram_tensor("bm1", [22, 1], F32, kind="ExternalInput").ap()
    b2p_d = nc.dram_tensor("b2pad", [1, CPAD], DTM, kind="ExternalInput").ap()
    e4_d = nc.dram_tensor("e4", [NSEG, 4 * 256], DTM, kind="ExternalInput").ap()
    ones_d = nc.dram_tensor("ones", [1, 128], DTM, kind="ExternalInput").ap()
    out_d = nc.dram_tensor("out", [ngp * 256, C], F32, kind="ExternalOutput").ap()

    RELU = mybir.ActivationFunctionType.Relu
    COPY = mybir.ActivationFunctionType.Copy

    with TileContext(nc) as tc:
        with (
            tc.tile_pool(name="const", bufs=1) as constp,
            tc.tile_pool(name="featp", bufs=feat_bufs) as featp,
            tc.tile_pool(name="meansp", bufs=8) as meansp,
            tc.tile_pool(name="diffp", bufs=2) as diffp,
            tc.tile_pool(name="actp", bufs=2) as actp,
            tc.tile_pool(name="osbp", bufs=3) as osbp,
            tc.tile_pool(name="mpsum", bufs=2, space="PSUM") as mpsum,
            tc.tile_pool(name="dpsum", bufs=2, space="PSUM") as dpsum,
            tc.tile_pool(name="hpsum", bufs=2, space="PSUM") as hpsum,
            tc.tile_pool(name="opsum", bufs=2, space="PSUM") as opsum,
        ):
            ohT_sb = constp.tile([128, rpc * tch * NSEG], DT1, tag="c_ohT")
            nc.gpsimd.dma_start(out=ohT_sb, in_=ohT_d)
            icnt_sb = constp.tile([NSEG, rpc], F32, tag="c_icnt")
            nc.gpsimd.dma_start(out=icnt_sb, in_=icnt_d)
            w1_sb = constp.tile([128, HC * C], DTM, tag="c_w1")
            nc.gpsimd.dma_start(out=w1_sb, in_=w1p_d)
            wm0_sb = constp.tile([128, C], DTM, tag="c_wm0")
            nc.gpsimd.dma_start(out=wm0_sb, in_=wm0_d)
            wm1_sb = constp.tile([22, C], DTM, tag="c_wm1")
            nc.gpsimd.dma_start(out=wm1_sb, in_=wm1_d)
            w20_sb = constp.tile([128, CPAD], DTM, tag="c_w20")
            nc.gpsimd.dma_start(out=w20_sb, in_=w20_d)
            w21_sb = constp.tile([22, CPAD], DTM, tag="c_w21")
            nc.gpsimd.dma_start(out=w21_sb, in_=w21_d)
            b1_sb = []
            for ci, (coff, csz) in enumerate(CC):
                t = constp.tile([csz, 1], F32, tag=f"c_b1_{ci}")
                nc.gpsimd.dma_start(out=t, in_=(b1c0_d, b1c1_d)[ci])
                b1_sb.append(t)
            bm_sb = []
            for ci, (coff, csz) in enumerate(CC):
                t = constp.tile([csz, 1], F32, tag=f"c_bm_{ci}")
                nc.gpsimd.dma_start(out=t, in_=(bm0_d, bm1_d)[ci])
                bm_sb.append(t)
            b2p_sb = constp.tile([1, CPAD], DTM, tag="c_b2")
            nc.gpsimd.dma_start(out=b2p_sb, in_=b2p_d)
            e4_sb = constp.tile([NSEG, 4 * 256], DTM, tag="c_e4")
            nc.gpsimd.dma_start(out=e4_sb, in_=e4_d)
            ones_sb = constp.tile([1, 128], DTM, tag="c_ones")
            nc.gpsimd.dma_start(out=ones_sb, in_=ones_d)

            for gp in range(ngp):
                # ---- stage 1: segment means for 4 batch rows ----
                means = []
                for r4 in range(4):
                    row = gp * 4 + r4
                    feat = featp.tile([128, tch, H], DT1, tag="feat")
                    dma_eng = nc.sync if (row % 2 == 0) else nc.scalar
                    dma_eng.dma_start(
                        out=feat,
                        in_=feats_d[row].rearrange("(t p) h -> p t h", p=128),
                    )
                    m = meansp.tile([NSEG, H], DTM, tag="means")
                    for half in range(2):
                        mp = mpsum.tile([NSEG, HHALF], F32, tag="mp")
                        for t in range(tch):
                            nc.tensor.matmul(
                                mp,
                                ohT_sb[:, ds(row * tch * NSEG + t * NSEG, NSEG)],
                                feat[:, t, ds(half * HHALF, HHALF)],
                                start=(t == 0),
                                stop=(t == tch - 1),
                            )
                        nc.scalar.activation(
                            m[:, ds(half * HHALF, HHALF)], mp, COPY,
                            scale=icnt_sb[:, ds(row, 1)],
                        )
                    means.append(m)

                # ---- pairwise diff (fused transpose): diffT = means^T @ E4 ----
                diff = diffp.tile([128, HC, 256], DTM, tag="diff")
                for hc in range(HC):
                    dp = dpsum.tile([128, 256], F32, tag="dp")
                    for r4 in range(4):
                        nc.tensor.matmul(
                            dp,
                            means[r4][:, ds(hc * 128, 128)],
                            e4_sb[:, ds(r4 * 256, 256)],
                            start=(r4 == 0),
                            stop=(r4 == 3),
                        )
                    nc.vector.tensor_copy(diff[:, hc, :], dp)

                # ---- mm1: h1T = relu(W1^T @ diffT + b1) ----
                h1 = []
                for ci, (coff, csz) in enumerate(CC):
                    hp = hpsum.tile([csz, 256], F32, tag="hp")
                    for hc in range(HC):
                        nc.tensor.matmul(
                            hp,
                            w1_sb[:, ds(hc * C + coff, csz)],
                            diff[:, hc, :],
                            start=(hc == 0),
                            stop=(hc == HC - 1),
                        )
                    hs = actp.tile([csz, 256], DTM, tag=f"h1s{ci}")
                    nc.scalar.activation(hs, hp, RELU, bias=b1_sb[ci])
                    h1.append(hs)

                # ---- mm2: h2T = relu(Wm^T @ h1T + bm) ----
                h2 = []
                for ci, (coff, csz) in enumerate(CC):
                    hp = hpsum.tile([csz, 256], F32, tag="hp")
                    nc.tensor.matmul(hp, wm0_sb[:, ds(coff, csz)],
                                     h1[0], start=True, stop=False)
                    nc.tensor.matmul(hp, wm1_sb[:, ds(coff, csz)],
                                     h1[1], start=False, stop=True)
                    hs = actp.tile([csz, 256], DTM, tag=f"h2s{ci}")
                    nc.scalar.activation(hs, hp, RELU, bias=bm_sb[ci])
                    h2.append(hs)

                # ---- mm3: out = h2 @ W2 + b2, natural [rows, c] layout ----
                for rs in range(2):
                    op = opsum.tile([128, CPAD], F32, tag="op")
                    nc.tensor.matmul(op, h2[0][:, ds(rs * 128, 128)],
                                     w20_sb, start=True, stop=False)
                    nc.tensor.matmul(op, h2[1][:, ds(rs * 128, 128)],
                                     w21_sb, start=False, stop=False)
                    nc.tensor.matmul(op, ones_sb,
                                     b2p_sb, start=False, stop=True)
                    osb = osbp.tile([128, C], F32, tag="osb")
                    nc.vector.tensor_copy(osb, op[:, 0:C])
                    nc.scalar.dma_start(
                        out=out_d[ds(gp * 256 + rs * 128, 128), :], in_=osb
                    )

    # TRN2 allows at most 1 sync wait per instruction (2 on event semaphores).
    # Tile can emit more; split them the same way Bacc.compile() does.
    import bass_rust as _bass_rust
    _bass_rust.move_matmul_waits_to_ldweights(nc.m)
    _bass_rust.generate_event_semaphores(nc)
    return nc


def host_prep(output_ids, features, W1, b1, Wm, bm, W2, b2, rpc=RPC, tch=TCH):
    """Build per-core input maps. Heavy data (features) is passed as-is;
    the tiny one-hot/count/weight tensors are repacked for device layout."""
    S_ = tch * 128
    ids = np.asarray(output_ids)
    nrows = ids.shape[0]
    ncores = nrows // rpc
    feats = np.ascontiguousarray(np.asarray(features, dtype=np.float32))

    is_sep = ids == SEP_ID
    seg = np.cumsum(is_sep.astype(np.int64), axis=1)
    valid = ~is_sep
    oh = ((seg[:, :, None] == np.arange(NSEG)[None, None, :]) & valid[:, :, None])
    oh = oh.astype(np.float32)                        # [B, S, 8]
    counts = oh.sum(axis=1)                           # [B, 8]
    icnt_full = (1.0 / np.maximum(counts, 1.0)).astype(np.float32)

    # E4 [8, r4, g2, b2, i, j]: column (g2,b2,i,j) of 4-row block, row-chunk r4
    eye = np.eye(NSEG, dtype=np.float32)
    base = eye[:, :, None] - eye[:, None, :]          # [n, i, j]
    e4 = np.zeros((NSEG, 4, 2, 2, NSEG, NSEG), np.float32)
    for r4 in range(4):
        e4[:, r4, r4 // 2, r4 % 2, :, :] = base
    e4 = np.ascontiguousarray(e4.reshape(NSEG, 4 * 256))

    W1 = np.asarray(W1, np.float32)
    Wm = np.asarray(Wm, np.float32)
    W2 = np.asarray(W2, np.float32)
    b1 = np.asarray(b1, np.float32)
    bm = np.asarray(bm, np.float32)
    b2 = np.asarray(b2, np.float32)

    w1p = np.ascontiguousarray(
        W1.reshape(HC, 128, C).transpose(1, 0, 2).reshape(128, HC * C))
    wm0 = np.ascontiguousarray(Wm[:128])
    wm1 = np.ascontiguousarray(Wm[128:])
    w2pad = np.zeros((C, CPAD), np.float32)
    w2pad[:, :C] = W2
    w20 = np.ascontiguousarray(w2pad[:128])
    w21 = np.ascontiguousarray(w2pad[128:])
    b2pad = np.zeros((1, CPAD), np.float32)
    b2pad[0, :C] = b2
    b1c0 = np.ascontiguousarray(b1[:128, None])
    b1c1 = np.ascontiguousarray(b1[128:, None])
    bm0 = np.ascontiguousarray(bm[:128, None])
    bm1 = np.ascontiguousarray(bm[128:, None])

    shared = dict(w1p=w1p, wm0=wm0, wm1=wm1, w20=w20, w21=w21,
                  b1c0=b1c0, b1c1=b1c1, bm0=bm0, bm1=bm1, b2pad=b2pad, e4=e4,
                  ones=np.ones((1, 128), np.float32))

    in_maps = []
    for c in range(ncores):
        rows = slice(c * rpc, (c + 1) * rpc)
        ohT = np.ascontiguousarray(
            oh[rows].reshape(rpc, tch, 128, NSEG)
            .transpose(2, 0, 1, 3).reshape(128, rpc * tch * NSEG))
        icnt = np.ascontiguousarray(icnt_full[rows].T)
        in_maps.append(dict(
            features=np.ascontiguousarray(feats[rows]),
            ohT=ohT, icnt=icnt, **shared))
    return in_maps


def gather_output(core_outs, rpc=RPC):
    """[ngp*256, C] per core -> [8, 8, B, C]."""
    ncores = len(core_outs)
    ngp = rpc // 4
    full = np.empty((NSEG, NSEG, ncores * rpc, C), np.float32)
    for c, o in enumerate(core_outs):
        o = o.reshape(ngp, 2, 2, NSEG, NSEG, C)       # gp, g2, b2, i, j, c
        o = o.transpose(3, 4, 0, 1, 2, 5).reshape(NSEG, NSEG, rpc, C)
        full[:, :, c * rpc:(c + 1) * rpc, :] = o
    return full


_NC_CACHE = {}


def _get_program():
    key = (RPC, TCH, F32R_STAGE1, F32R_MLP)
    if key not in _NC_CACHE:
        _NC_CACHE[key] = build_program()
    return _NC_CACHE[key]


def run(inputs, trace=False, trace_cores=None):
    nc = _get_program()
    in_maps = host_prep(**inputs)
    res = run_bass_kernel_spmd(
        nc, in_maps, core_ids=list(range(NCORES)),
        trace=trace, trace_cores=trace_cores,
    )
    out = gather_output([r["out"] for r in res.results])
    return out, res


def kernel(**inputs):
    out, _ = run(inputs, trace=False)
    return out



# revision 4
# speedup vs baseline: 1.4127x; 1.4127x over previous
"""Trainium2 Bass kernel: segment-mean over token segments + pairwise-diff edge MLP.

Reference computation (per batch row b):
  seg = cumsum(ids == 3); valid = ids != 3
  means[n] = mean of features[s] over tokens with seg==n & valid (n < 8), 0-count -> sum/1
  diff[i,j] = means[i] - means[j]                          # [8,8,H]
  out[i,j]  = relu(relu(diff @ W1 + b1) @ Wm + bm) @ W2 + b2   # [8,8,150]

Distribution: data-parallel over batch B=128 across 8 NeuronCores (16 rows/core),
tiny MLP weights replicated, no cross-core communication.

The kernel is HBM-bandwidth-bound (features = 50 MB/core in fp32). Features are
cast to bf16 on the host (tolerance is 2e-2; bf16 rounding contributes ~4e-3),
halving HBM traffic, and laid out so each SBUF partition line is one contiguous
12 KB DMA descriptor (token s -> partition s//8, chunk s%8).

Device algorithm per core:
  stage1: means [8seg, 768] per row via TensorE: onehot*(1/count) stationary
          (host-precomputed, bf16) x features (moving, bf16) accumulated over
          8 token chunks; plain cast eviction PSUM->SBUF (scalar/vector split).
  diff:   one matmul per (group-of-4-rows, h-chunk): diffT = means^T @ E4 where
          E4 is a constant +-1 selection matrix -> fuses the transpose AND the
          pairwise difference. Output columns = (g2, b2, i, j) = 256 per 4 rows.
  MLP:    transposed matmuls, contraction dim on partitions, c-dim split 128+22.
          Biases b1/bm applied as per-partition activation bias (c on
          partitions); b2 added via a K=1 matmul with a ones row. Final out is
          [rows, 150] fp32.
"""

import sys

import numpy as np
import ml_dtypes

if "/opt/trn_rl_repo" not in sys.path:
    sys.path.insert(0, "/opt/trn_rl_repo")

import concourse.bass as bass
import concourse.mybir as mybir
from concourse.bass import ds
from concourse.bass_utils import run_bass_kernel_spmd
from concourse.tile import TileContext

B, S, H, C = 128, 1024, 768, 150
NSEG = 8
SEP_ID = 3
NCORES = 8
RPC = B // NCORES  # 16 rows per core
TCH = S // 128     # 8 token chunks
HC = H // 128      # 6 hidden chunks
HHALF = 384        # H split for PSUM bank limit
CC = ((0, 128), (128, 22))  # c-dim (150) chunks
CPAD = 256         # final free dim padded to 256

F32 = mybir.dt.float32
BF16 = mybir.dt.bfloat16
NPBF16 = ml_dtypes.bfloat16


def build_program(rpc=RPC, tch=TCH, feat_bufs=8):
    ngp = rpc // 4  # groups of 4 batch rows -> 256 output rows each
    nc = bass.Bass("TRN2", target_bir_lowering=False, debug=False)

    feats_d = nc.dram_tensor("features", [rpc, tch * 128, H], BF16,
                             kind="ExternalInput").ap()
    ohT_d = nc.dram_tensor("ohT", [128, rpc * tch * NSEG], BF16,
                           kind="ExternalInput").ap()
    w1p_d = nc.dram_tensor("w1p", [128, HC * C], BF16, kind="ExternalInput").ap()
    wm0_d = nc.dram_tensor("wm0", [128, C], BF16, kind="ExternalInput").ap()
    wm1_d = nc.dram_tensor("wm1", [22, C], BF16, kind="ExternalInput").ap()
    w20_d = nc.dram_tensor("w20", [128, CPAD], BF16, kind="ExternalInput").ap()
    w21_d = nc.dram_tensor("w21", [22, CPAD], BF16, kind="ExternalInput").ap()
    b1c0_d = nc.dram_tensor("b1c0", [128, 1], F32, kind="ExternalInput").ap()
    b1c1_d = nc.dram_tensor("b1c1", [22, 1], F32, kind="ExternalInput").ap()
    bm0_d = nc.dram_tensor("bm0", [128, 1], F32, kind="ExternalInput").ap()
    bm1_d = nc.dram_tensor("bm1", [22, 1], F32, kind="ExternalInput").ap()
    b2p_d = nc.dram_tensor("b2pad", [1, CPAD], BF16, kind="ExternalInput").ap()
    e4_d = nc.dram_tensor("e4", [NSEG, 4 * 256], BF16, kind="ExternalInput").ap()
    ones_d = nc.dram_tensor("ones", [1, 128], BF16, kind="ExternalInput").ap()
    out_d = nc.dram_tensor("out", [ngp * 256, C], F32, kind="ExternalOutput").ap()

    RELU = mybir.ActivationFunctionType.Relu
    COPY = mybir.ActivationFunctionType.Copy

    with TileContext(nc) as tc:
        with (
            tc.tile_pool(name="const", bufs=1) as constp,
            tc.tile_pool(name="featp", bufs=feat_bufs) as featp,
            tc.tile_pool(name="meansp", bufs=8) as meansp,
            tc.tile_pool(name="diffp", bufs=2) as diffp,
            tc.tile_pool(name="actp", bufs=2) as actp,
            tc.tile_pool(name="osbp", bufs=3) as osbp,
            tc.tile_pool(name="mpsum", bufs=4, space="PSUM") as mpsum,
            tc.tile_pool(name="dpsum", bufs=2, space="PSUM") as dpsum,
            tc.tile_pool(name="hpsum", bufs=2, space="PSUM") as hpsum,
        ):
            # ohT first on the sync queue so row-0 matmuls unblock ASAP;
            # remaining consts spread over scalar/vector/gpsimd queues.
            ohT_sb = constp.tile([128, rpc * tch * NSEG], BF16, tag="c_ohT")
            nc.sync.dma_start(out=ohT_sb, in_=ohT_d)
            e4_sb = constp.tile([NSEG, 4 * 256], BF16, tag="c_e4")
            nc.gpsimd.dma_start(out=e4_sb, in_=e4_d)
            w1_sb = constp.tile([128, HC * C], BF16, tag="c_w1")
            nc.scalar.dma_start(out=w1_sb, in_=w1p_d)
            wm0_sb = constp.tile([128, C], BF16, tag="c_wm0")
            nc.scalar.dma_start(out=wm0_sb, in_=wm0_d)
            wm1_sb = constp.tile([22, C], BF16, tag="c_wm1")
            nc.scalar.dma_start(out=wm1_sb, in_=wm1_d)
            w20_sb = constp.tile([128, CPAD], BF16, tag="c_w20")
            nc.scalar.dma_start(out=w20_sb, in_=w20_d)
            w21_sb = constp.tile([22, CPAD], BF16, tag="c_w21")
            nc.scalar.dma_start(out=w21_sb, in_=w21_d)
            b2p_sb = constp.tile([1, CPAD], BF16, tag="c_b2")
            nc.scalar.dma_start(out=b2p_sb, in_=b2p_d)
            ones_sb = constp.tile([1, 128], BF16, tag="c_ones")
            nc.scalar.dma_start(out=ones_sb, in_=ones_d)
            b1_sb = []
            for ci, (coff, csz) in enumerate(CC):
                t = constp.tile([csz, 1], F32, tag=f"c_b1_{ci}")
                nc.gpsimd.dma_start(out=t, in_=(b1c0_d, b1c1_d)[ci])
                b1_sb.append(t)
            bm_sb = []
            for ci, (coff, csz) in enumerate(CC):
                t = constp.tile([csz, 1], F32, tag=f"c_bm_{ci}")
                nc.gpsimd.dma_start(out=t, in_=(bm0_d, bm1_d)[ci])
                bm_sb.append(t)

            for gp in range(ngp):
                # ---- stage 1: segment means for 4 batch rows ----
                means = []
                for r4 in range(4):
                    row = gp * 4 + r4
                    feat = featp.tile([128, tch * H], BF16, tag="feat")
                    dma_eng = nc.sync if (row % 2 == 0) else nc.gpsimd
                    dma_eng.dma_start(
                        out=feat,
                        in_=feats_d[row].rearrange("(p t) h -> p (t h)", t=tch),
                    )
                    m = meansp.tile([NSEG, H], BF16, tag="means")
                    for half in range(2):
                        mp = mpsum.tile([NSEG, HHALF], F32, tag="mp")
                        for t in range(tch):
                            nc.tensor.matmul(
                                mp,
                                ohT_sb[:, ds((row * tch + t) * NSEG, NSEG)],
                                feat[:, ds(t * H + half * HHALF, HHALF)],
                                start=(t == 0),
                                stop=(t == tch - 1),
                            )
                        if half == 0:
                            nc.scalar.activation(
                                m[:, ds(0, HHALF)], mp, COPY)
                        else:
                            nc.vector.tensor_copy(
                                m[:, ds(HHALF, HHALF)], mp)
                    means.append(m)

                # ---- pairwise diff (fused transpose): diffT = means^T @ E4 ----
                diff = diffp.tile([128, HC, 256], BF16, tag="diff")
                for hc in range(HC):
                    dp = dpsum.tile([128, 256], F32, tag="dp")
                    for r4 in range(4):
                        nc.tensor.matmul(
                            dp,
                            means[r4][:, ds(hc * 128, 128)],
                            e4_sb[:, ds(r4 * 256, 256)],
                            start=(r4 == 0),
                            stop=(r4 == 3),
                        )
                    nc.vector.tensor_copy(diff[:, hc, :], dp)

                # ---- mm1: h1T = relu(W1^T @ diffT + b1) ----
                h1 = []
                for ci, (coff, csz) in enumerate(CC):
                    hp = hpsum.tile([csz, 256], F32, tag="hp")
                    for hc in range(HC):
                        nc.tensor.matmul(
                            hp,
                            w1_sb[:, ds(hc * C + coff, csz)],
                            diff[:, hc, :],
                            start=(hc == 0),
                            stop=(hc == HC - 1),
                        )
                    hs = actp.tile([csz, 256], BF16, tag=f"h1s{ci}")
                    nc.scalar.activation(hs, hp, RELU, bias=b1_sb[ci])
                    h1.append(hs)

                # ---- mm2: h2T = relu(Wm^T @ h1T + bm) ----
                h2 = []
                for ci, (coff, csz) in enumerate(CC):
                    hp = hpsum.tile([csz, 256], F32, tag="hp")
                    nc.tensor.matmul(hp, wm0_sb[:, ds(coff, csz)],
                                     h1[0], start=True, stop=False)
                    nc.tensor.matmul(hp, wm1_sb[:, ds(coff, csz)],
                                     h1[1], start=False, stop=True)
                    hs = actp.tile([csz, 256], BF16, tag=f"h2s{ci}")
                    nc.scalar.activation(hs, hp, RELU, bias=bm_sb[ci])
                    h2.append(hs)

                # ---- mm3: out = h2 @ W2 + b2, natural [rows, c] layout ----
                for rs in range(2):
                    op = dpsum.tile([128, 256], F32, tag="dp")
                    nc.tensor.matmul(op, h2[0][:, ds(rs * 128, 128)],
                                     w20_sb, start=True, stop=False)
                    nc.tensor.matmul(op, h2[1][:, ds(rs * 128, 128)],
                                     w21_sb, start=False, stop=False)
                    nc.tensor.matmul(op, ones_sb,
                                     b2p_sb, start=False, stop=True)
                    osb = osbp.tile([128, C], F32, tag="osb")
                    nc.vector.tensor_copy(osb, op[:, 0:C])
                    nc.sync.dma_start(
                        out=out_d[ds(gp * 256 + rs * 128, 128), :], in_=osb
                    )

    # TRN2 allows at most 1 sync wait per instruction (2 on event semaphores).
    # Tile can emit more; split them the same way Bacc.compile() does.
    import bass_rust as _bass_rust
    _bass_rust.move_matmul_waits_to_ldweights(nc.m)
    _bass_rust.generate_event_semaphores(nc)
    return nc


def host_prep(output_ids, features, W1, b1, Wm, bm, W2, b2, rpc=RPC, tch=TCH):
    """Build per-core input maps. Features are cast to bf16 and kept in natural
    [rows, S, H] layout (token s -> partition s//8, chunk s%8 on device); the
    tiny one-hot/weight tensors are repacked for device layout."""
    ids = np.asarray(output_ids)
    nrows = ids.shape[0]
    ncores = nrows // rpc
    feats = np.asarray(features, dtype=np.float32).astype(NPBF16)

    is_sep = ids == SEP_ID
    seg = np.cumsum(is_sep.astype(np.int64), axis=1)
    valid = ~is_sep
    oh = ((seg[:, :, None] == np.arange(NSEG)[None, None, :]) & valid[:, :, None])
    oh = oh.astype(np.float32)                        # [B, S, 8]
    counts = oh.sum(axis=1)                           # [B, 8]
    oh *= (1.0 / np.maximum(counts, 1.0))[:, None, :]  # fold mean scale

    # E4 [8, r4, g2, b2, i, j]: column (g2,b2,i,j) of 4-row block, row-chunk r4
    eye = np.eye(NSEG, dtype=np.float32)
    base = eye[:, :, None] - eye[:, None, :]          # [n, i, j]
    e4 = np.zeros((NSEG, 4, 2, 2, NSEG, NSEG), np.float32)
    for r4 in range(4):
        e4[:, r4, r4 // 2, r4 % 2, :, :] = base
    e4 = np.ascontiguousarray(e4.reshape(NSEG, 4 * 256)).astype(NPBF16)

    W1 = np.asarray(W1, np.float32)
    Wm = np.asarray(Wm, np.float32)
    W2 = np.asarray(W2, np.float32)
    b1 = np.asarray(b1, np.float32)
    bm = np.asarray(bm, np.float32)
    b2 = np.asarray(b2, np.float32)

    w1p = np.ascontiguousarray(
        W1.reshape(HC, 128, C).transpose(1, 0, 2).reshape(128, HC * C)
    ).astype(NPBF16)
    wm0 = np.ascontiguousarray(Wm[:128]).astype(NPBF16)
    wm1 = np.ascontiguousarray(Wm[128:]).astype(NPBF16)
    w2pad = np.zeros((C, CPAD), np.float32)
    w2pad[:, :C] = W2
    w20 = np.ascontiguousarray(w2pad[:128]).astype(NPBF16)
    w21 = np.ascontiguousarray(w2pad[128:]).astype(NPBF16)
    b2pad = np.zeros((1, CPAD), np.float32)
    b2pad[0, :C] = b2
    b2pad = b2pad.astype(NPBF16)
    b1c0 = np.ascontiguousarray(b1[:128, None])
    b1c1 = np.ascontiguousarray(b1[128:, None])
    bm0 = np.ascontiguousarray(bm[:128, None])
    bm1 = np.ascontiguousarray(bm[128:, None])

    shared = dict(w1p=w1p, wm0=wm0, wm1=wm1, w20=w20, w21=w21,
                  b1c0=b1c0, b1c1=b1c1, bm0=bm0, bm1=bm1, b2pad=b2pad, e4=e4,
                  ones=np.ones((1, 128), NPBF16))

    in_maps = []
    for c in range(ncores):
        rows = slice(c * rpc, (c + 1) * rpc)
        # token s -> (partition p=s//8, chunk t=s%8); col (r, t, n)
        ohT = np.ascontiguousarray(
            oh[rows].reshape(rpc, 128, tch, NSEG)
            .transpose(1, 0, 2, 3).reshape(128, rpc * tch * NSEG)
        ).astype(NPBF16)
        in_maps.append(dict(
            features=np.ascontiguousarray(feats[rows]),
            ohT=ohT, **shared))
    return in_maps


def gather_output(core_outs, rpc=RPC):
    """[ngp*256, C] per core -> [8, 8, B, C]."""
    ncores = len(core_outs)
    ngp = rpc // 4
    full = np.empty((NSEG, NSEG, ncores * rpc, C), np.float32)
    for c, o in enumerate(core_outs):
        o = o.reshape(ngp, 2, 2, NSEG, NSEG, C)       # gp, g2, b2, i, j, c
        o = o.transpose(3, 4, 0, 1, 2, 5).reshape(NSEG, NSEG, rpc, C)
        full[:, :, c * rpc:(c + 1) * rpc, :] = o
    return full


_NC_CACHE = {}


def _get_program():
    key = (RPC, TCH)
    if key not in _NC_CACHE:
        _NC_CACHE[key] = build_program()
    return _NC_CACHE[key]


def run(inputs, trace=False, trace_cores=None):
    nc = _get_program()
    in_maps = host_prep(**inputs)
    res = run_bass_kernel_spmd(
        nc, in_maps, core_ids=list(range(NCORES)),
        trace=trace, trace_cores=trace_cores,
    )
    out = gather_output([r["out"] for r in res.results])
    return out, res


def kernel(**inputs):
    out, _ = run(inputs, trace=False)
    return out


# revision 6
# speedup vs baseline: 1.6383x; 1.1597x over previous
"""Trainium2 Bass kernel: segment-mean over token segments + pairwise-diff edge MLP.

Reference computation (per batch row b):
  seg = cumsum(ids == 3); valid = ids != 3
  means[n] = mean of features[s] over tokens with seg==n & valid (n < 8), 0-count -> sum/1
  diff[i,j] = means[i] - means[j]                          # [8,8,H]
  out[i,j]  = relu(relu(diff @ W1 + b1) @ Wm + bm) @ W2 + b2   # [8,8,150]

Distribution: data-parallel over batch B=128 across 8 NeuronCores (16 rows/core),
tiny MLP weights replicated, no cross-core communication.

The kernel is HBM-bandwidth-bound (features = 50 MB/core in fp32). Features are
cast to bf16 on the host (tolerance is 2e-2; bf16 rounding contributes ~4e-3),
halving HBM traffic, and laid out so each SBUF partition line is one contiguous
12 KB DMA descriptor (token s -> partition s//8, chunk s%8).

Device algorithm per core:
  stage1: means [8seg, 768] per row via TensorE: onehot*(1/count) stationary
          (host-precomputed, bf16) x features (moving, bf16) accumulated over
          8 token chunks; plain cast eviction PSUM->SBUF (scalar/vector split).
  diff:   one matmul per (group-of-4-rows, h-chunk): diffT = means^T @ E4 where
          E4 is a constant +-1 selection matrix -> fuses the transpose AND the
          pairwise difference. Output columns = (g2, b2, i, j) = 256 per 4 rows.
  MLP:    transposed matmuls, contraction dim on partitions, c-dim split 128+22.
          Biases b1/bm applied as per-partition activation bias (c on
          partitions); b2 added via a K=1 matmul with a ones row. Final out is
          [rows, 150] fp32.
"""

import sys

import numpy as np
import ml_dtypes

if "/opt/trn_rl_repo" not in sys.path:
    sys.path.insert(0, "/opt/trn_rl_repo")

import concourse.bass as bass
import concourse.mybir as mybir
from concourse.bass import ds
from concourse.bass_utils import run_bass_kernel_spmd
from concourse.tile import TileContext

B, S, H, C = 128, 1024, 768, 150
NSEG = 8
SEP_ID = 3
NCORES = 8
RPC = B // NCORES  # 16 rows per core
TCH = S // 128     # 8 token chunks
HC = H // 128      # 6 hidden chunks
HHALF = 384        # H split for PSUM bank limit
CC = ((0, 128), (128, 22))  # c-dim (150) chunks
CPAD = 256         # final free dim padded to 256

F32 = mybir.dt.float32
BF16 = mybir.dt.bfloat16
NPBF16 = ml_dtypes.bfloat16


def build_program(rpc=RPC, tch=TCH, feat_bufs=8):
    ngp = rpc // 4  # groups of 4 batch rows -> 256 output rows each
    nc = bass.Bass("TRN2", target_bir_lowering=False, debug=False)

    feats_d = nc.dram_tensor("features", [rpc, tch * 128, H], BF16,
                             kind="ExternalInput").ap()
    ohT_d = nc.dram_tensor("ohT", [128, rpc * tch * NSEG], BF16,
                           kind="ExternalInput").ap()
    w1p_d = nc.dram_tensor("w1p", [128, HC * C], BF16, kind="ExternalInput").ap()
    wm0_d = nc.dram_tensor("wm0", [128, C], BF16, kind="ExternalInput").ap()
    wm1_d = nc.dram_tensor("wm1", [22, C], BF16, kind="ExternalInput").ap()
    w20_d = nc.dram_tensor("w20", [128, CPAD], BF16, kind="ExternalInput").ap()
    w21_d = nc.dram_tensor("w21", [22, CPAD], BF16, kind="ExternalInput").ap()
    b1c0_d = nc.dram_tensor("b1c0", [128, 1], F32, kind="ExternalInput").ap()
    b1c1_d = nc.dram_tensor("b1c1", [22, 1], F32, kind="ExternalInput").ap()
    bm0_d = nc.dram_tensor("bm0", [128, 1], F32, kind="ExternalInput").ap()
    bm1_d = nc.dram_tensor("bm1", [22, 1], F32, kind="ExternalInput").ap()
    b2p_d = nc.dram_tensor("b2pad", [1, CPAD], BF16, kind="ExternalInput").ap()
    e4_d = nc.dram_tensor("e4", [NSEG, 4 * 256], BF16, kind="ExternalInput").ap()
    ones_d = nc.dram_tensor("ones", [1, 128], BF16, kind="ExternalInput").ap()
    out_d = nc.dram_tensor("out", [ngp * 256, C], F32, kind="ExternalOutput").ap()

    RELU = mybir.ActivationFunctionType.Relu
    COPY = mybir.ActivationFunctionType.Copy

    with TileContext(nc) as tc:
        with (
            tc.tile_pool(name="const", bufs=1) as constp,
            tc.tile_pool(name="featp", bufs=feat_bufs) as featp,
            tc.tile_pool(name="meansp", bufs=8) as meansp,
            tc.tile_pool(name="diffp", bufs=2) as diffp,
            tc.tile_pool(name="actp", bufs=2) as actp,
            tc.tile_pool(name="osbp", bufs=3) as osbp,
            tc.tile_pool(name="mpsum", bufs=4, space="PSUM") as mpsum,
            tc.tile_pool(name="dpsum", bufs=2, space="PSUM") as dpsum,
            tc.tile_pool(name="hpsum", bufs=2, space="PSUM") as hpsum,
        ):
            # The sync HWDGE queue is dedicated to feature streaming (the
            # critical path); everything else issues from the scalar queue.
            # gpsimd dma_start uses the slow SWDGE path with DRAINs - avoid.
            ohT_sb = constp.tile([128, rpc * tch * NSEG], BF16, tag="c_ohT")
            nc.sync.dma_start(out=ohT_sb, in_=ohT_d)
            e4_sb = constp.tile([NSEG, 4 * 256], BF16, tag="c_e4")
            nc.scalar.dma_start(out=e4_sb, in_=e4_d)
            w1_sb = constp.tile([128, HC * C], BF16, tag="c_w1")
            nc.scalar.dma_start(out=w1_sb, in_=w1p_d)
            wm0_sb = constp.tile([128, C], BF16, tag="c_wm0")
            nc.scalar.dma_start(out=wm0_sb, in_=wm0_d)
            wm1_sb = constp.tile([22, C], BF16, tag="c_wm1")
            nc.scalar.dma_start(out=wm1_sb, in_=wm1_d)
            w20_sb = constp.tile([128, CPAD], BF16, tag="c_w20")
            nc.scalar.dma_start(out=w20_sb, in_=w20_d)
            w21_sb = constp.tile([22, CPAD], BF16, tag="c_w21")
            nc.scalar.dma_start(out=w21_sb, in_=w21_d)
            b2p_sb = constp.tile([1, CPAD], BF16, tag="c_b2")
            nc.scalar.dma_start(out=b2p_sb, in_=b2p_d)
            ones_sb = constp.tile([1, 128], BF16, tag="c_ones")
            nc.scalar.dma_start(out=ones_sb, in_=ones_d)
            b1_sb = []
            for ci, (coff, csz) in enumerate(CC):
                t = constp.tile([csz, 1], F32, tag=f"c_b1_{ci}")
                nc.scalar.dma_start(out=t, in_=(b1c0_d, b1c1_d)[ci])
                b1_sb.append(t)
            bm_sb = []
            for ci, (coff, csz) in enumerate(CC):
                t = constp.tile([csz, 1], F32, tag=f"c_bm_{ci}")
                nc.scalar.dma_start(out=t, in_=(bm0_d, bm1_d)[ci])
                bm_sb.append(t)

            for gp in range(ngp):
                # ---- stage 1: segment means for 4 batch rows ----
                means = []
                for r4 in range(4):
                    row = gp * 4 + r4
                    # [p, t, half, 384]; two DMAs split by H-half so the
                    # half-0 accumulation starts as soon as its columns land
                    feat = featp.tile([128, tch, 2, HHALF], BF16, tag="feat")
                    fd = feats_d[row].rearrange(
                        "(p t) (hh hc) -> p t hh hc", t=tch, hh=2)
                    for hh in range(2):
                        nc.sync.dma_start(
                            out=feat[:, :, hh, :], in_=fd[:, :, hh, :])
                    m = meansp.tile([NSEG, H], BF16, tag="means")
                    for half in range(2):
                        mp = mpsum.tile([NSEG, HHALF], F32, tag="mp")
                        for t in range(tch):
                            nc.tensor.matmul(
                                mp,
                                ohT_sb[:, ds((row * tch + t) * NSEG, NSEG)],
                                feat[:, t, half, :],
                                start=(t == 0),
                                stop=(t == tch - 1),
                            )
                        if half == 0:
                            nc.scalar.activation(
                                m[:, ds(0, HHALF)], mp, COPY)
                        else:
                            nc.vector.tensor_copy(
                                m[:, ds(HHALF, HHALF)], mp)
                    means.append(m)

                # ---- pairwise diff (fused transpose): diffT = means^T @ E4 ----
                diff = diffp.tile([128, HC, 256], BF16, tag="diff")
                for hc in range(HC):
                    dp = dpsum.tile([128, 256], F32, tag="dp")
                    for r4 in range(4):
                        nc.tensor.matmul(
                            dp,
                            means[r4][:, ds(hc * 128, 128)],
                            e4_sb[:, ds(r4 * 256, 256)],
                            start=(r4 == 0),
                            stop=(r4 == 3),
                        )
                    nc.vector.tensor_copy(diff[:, hc, :], dp)

                # ---- mm1: h1T = relu(W1^T @ diffT + b1) ----
                h1 = []
                for ci, (coff, csz) in enumerate(CC):
                    hp = hpsum.tile([csz, 256], F32, tag="hp")
                    for hc in range(HC):
                        nc.tensor.matmul(
                            hp,
                            w1_sb[:, ds(hc * C + coff, csz)],
                            diff[:, hc, :],
                            start=(hc == 0),
                            stop=(hc == HC - 1),
                        )
                    hs = actp.tile([csz, 256], BF16, tag=f"h1s{ci}")
                    nc.scalar.activation(hs, hp, RELU, bias=b1_sb[ci])
                    h1.append(hs)

                # ---- mm2: h2T = relu(Wm^T @ h1T + bm) ----
                h2 = []
                for ci, (coff, csz) in enumerate(CC):
                    hp = hpsum.tile([csz, 256], F32, tag="hp")
                    nc.tensor.matmul(hp, wm0_sb[:, ds(coff, csz)],
                                     h1[0], start=True, stop=False)
                    nc.tensor.matmul(hp, wm1_sb[:, ds(coff, csz)],
                                     h1[1], start=False, stop=True)
                    hs = actp.tile([csz, 256], BF16, tag=f"h2s{ci}")
                    nc.scalar.activation(hs, hp, RELU, bias=bm_sb[ci])
                    h2.append(hs)

                # ---- mm3: out = h2 @ W2 + b2, natural [rows, c] layout ----
                for rs in range(2):
                    op = dpsum.tile([128, 256], F32, tag="dp")
                    nc.tensor.matmul(op, h2[0][:, ds(rs * 128, 128)],
                                     w20_sb, start=True, stop=False)
                    nc.tensor.matmul(op, h2[1][:, ds(rs * 128, 128)],
                                     w21_sb, start=False, stop=False)
                    nc.tensor.matmul(op, ones_sb,
                                     b2p_sb, start=False, stop=True)
                    osb = osbp.tile([128, C], F32, tag="osb")
                    nc.vector.tensor_copy(osb, op[:, 0:C])
                    nc.scalar.dma_start(
                        out=out_d[ds(gp * 256 + rs * 128, 128), :], in_=osb
                    )

    # TRN2 allows at most 1 sync wait per instruction (2 on event semaphores).
    # Tile can emit more; split them the same way Bacc.compile() does.
    import bass_rust as _bass_rust
    _bass_rust.move_matmul_waits_to_ldweights(nc.m)
    _bass_rust.generate_event_semaphores(nc)
    return nc


def host_prep(output_ids, features, W1, b1, Wm, bm, W2, b2, rpc=RPC, tch=TCH):
    """Build per-core input maps. Features are cast to bf16 and kept in natural
    [rows, S, H] layout (token s -> partition s//8, chunk s%8 on device); the
    tiny one-hot/weight tensors are repacked for device layout."""
    ids = np.asarray(output_ids)
    nrows = ids.shape[0]
    ncores = nrows // rpc
    feats = np.asarray(features, dtype=np.float32).astype(NPBF16)

    is_sep = ids == SEP_ID
    seg = np.cumsum(is_sep.astype(np.int64), axis=1)
    valid = ~is_sep
    oh = ((seg[:, :, None] == np.arange(NSEG)[None, None, :]) & valid[:, :, None])
    oh = oh.astype(np.float32)                        # [B, S, 8]
    counts = oh.sum(axis=1)                           # [B, 8]
    oh *= (1.0 / np.maximum(counts, 1.0))[:, None, :]  # fold mean scale

    # E4 [8, r4, g2, b2, i, j]: column (g2,b2,i,j) of 4-row block, row-chunk r4
    eye = np.eye(NSEG, dtype=np.float32)
    base = eye[:, :, None] - eye[:, None, :]          # [n, i, j]
    e4 = np.zeros((NSEG, 4, 2, 2, NSEG, NSEG), np.float32)
    for r4 in range(4):
        e4[:, r4, r4 // 2, r4 % 2, :, :] = base
    e4 = np.ascontiguousarray(e4.reshape(NSEG, 4 * 256)).astype(NPBF16)

    W1 = np.asarray(W1, np.float32)
    Wm = np.asarray(Wm, np.float32)
    W2 = np.asarray(W2, np.float32)
    b1 = np.asarray(b1, np.float32)
    bm = np.asarray(bm, np.float32)
    b2 = np.asarray(b2, np.float32)

    w1p = np.ascontiguousarray(
        W1.reshape(HC, 128, C).transpose(1, 0, 2).reshape(128, HC * C)
    ).astype(NPBF16)
    wm0 = np.ascontiguousarray(Wm[:128]).astype(NPBF16)
    wm1 = np.ascontiguousarray(Wm[128:]).astype(NPBF16)
    w2pad = np.zeros((C, CPAD), np.float32)
    w2pad[:, :C] = W2
    w20 = np.ascontiguousarray(w2pad[:128]).astype(NPBF16)
    w21 = np.ascontiguousarray(w2pad[128:]).astype(NPBF16)
    b2pad = np.zeros((1, CPAD), np.float32)
    b2pad[0, :C] = b2
    b2pad = b2pad.astype(NPBF16)
    b1c0 = np.ascontiguousarray(b1[:128, None])
    b1c1 = np.ascontiguousarray(b1[128:, None])
    bm0 = np.ascontiguousarray(bm[:128, None])
    bm1 = np.ascontiguousarray(bm[128:, None])

    shared = dict(w1p=w1p, wm0=wm0, wm1=wm1, w20=w20, w21=w21,
                  b1c0=b1c0, b1c1=b1c1, bm0=bm0, bm1=bm1, b2pad=b2pad, e4=e4,
                  ones=np.ones((1, 128), NPBF16))

    in_maps = []
    for c in range(ncores):
        rows = slice(c * rpc, (c + 1) * rpc)
        # token s -> (partition p=s//8, chunk t=s%8); col (r, t, n)
        ohT = np.ascontiguousarray(
            oh[rows].reshape(rpc, 128, tch, NSEG)
            .transpose(1, 0, 2, 3).reshape(128, rpc * tch * NSEG)
        ).astype(NPBF16)
        in_maps.append(dict(
            features=np.ascontiguousarray(feats[rows]),
            ohT=ohT, **shared))
    return in_maps


def gather_output(core_outs, rpc=RPC):
    """[ngp*256, C] per core -> [8, 8, B, C]."""
    ncores = len(core_outs)
    ngp = rpc // 4
    full = np.empty((NSEG, NSEG, ncores * rpc, C), np.float32)
    for c, o in enumerate(core_outs):
        o = o.reshape(ngp, 2, 2, NSEG, NSEG, C)       # gp, g2, b2, i, j, c
        o = o.transpose(3, 4, 0, 1, 2, 5).reshape(NSEG, NSEG, rpc, C)
        full[:, :, c * rpc:(c + 1) * rpc, :] = o
    return full


_NC_CACHE = {}


def _get_program():
    key = (RPC, TCH)
    if key not in _NC_CACHE:
        _NC_CACHE[key] = build_program()
    return _NC_CACHE[key]


def run(inputs, trace=False, trace_cores=None):
    nc = _get_program()
    in_maps = host_prep(**inputs)
    res = run_bass_kernel_spmd(
        nc, in_maps, core_ids=list(range(NCORES)),
        trace=trace, trace_cores=trace_cores,
    )
    out = gather_output([r["out"] for r in res.results])
    return out, res


def kernel(**inputs):
    out, _ = run(inputs, trace=False)
    return out
